# revision 1
# baseline (speedup 1.0000x reference)
"""Trainium2 Bass kernel: negative squared-distance VQ codebook scores.

score[b,t,k] = -precision * ||x[b,t] - codebook[k]||^2
             = 2p * (x.c) - p * ||x||^2 - p * ||c||^2

Strategy (8 NeuronCores, data-parallel over B):
  - Each core gets 2 batches = 2048 (b,t) rows of x; codebook replicated.
  - On-chip: transpose x tiles (PE) to put D on partitions, scale by -2.
  - GEMM in float32r (full-rate fp32 mode): psum = -2*x.c, with ||c||^2
    folded in as a rank-1 update (ones ⊗ c_sq row).
  - Epilogue: out = (-p) * psum + (-p * x_sq)  per-partition scalars,
    alternated between ScalarE (activation) and VectorE (tensor_scalar).
"""

from contextlib import ExitStack

import numpy as np

import concourse.bass as bass
import concourse.tile as tile
from concourse import bacc, mybir
from concourse.bass_utils import run_bass_kernel_spmd
from concourse.masks import make_identity

B, T, D, K = 16, 1024, 256, 1024
N_CORES = 8
BT = B * T // N_CORES  # rows of x per core (2048)
P = 128                # partition tile
NT = BT // P           # bt tiles per core (16)
KH = 512               # k slice per matmul (fp32 moving-operand max)
DH = D // P            # d halves (2)
KT = K // P            # codebook row tiles (8)

F32 = mybir.dt.float32
F32R = mybir.dt.float32r
AF = mybir.ActivationFunctionType
OP = mybir.AluOpType


def _build_kernel(ctx: ExitStack, tc: tile.TileContext, x_in, cb_in, p_in, out):
    nc = tc.nc

    singles = ctx.enter_context(tc.tile_pool(name="singles", bufs=1))
    cbt_pool = ctx.enter_context(tc.tile_pool(name="cbt", bufs=1))

    ident = singles.tile([P, P], F32)
    make_identity(nc, ident)

    # precision broadcast to [128,1]; neg_p = -p
    p_bc = singles.tile([P, 1], F32)
    nc.gpsimd.dma_start(out=p_bc, in_=p_in.to_broadcast([P, 1]))
    neg_p = singles.tile([P, 1], F32)
    nc.vector.tensor_scalar_mul(neg_p, p_bc, -1.0)

    # memset can't emit fp32r; stage in f32 and copy (ALU ops can round)
    ones_col_f32 = singles.tile([P, 1], F32)
    nc.vector.memset(ones_col_f32, 1.0)
    ones_col = singles.tile([P, 1], F32R)
    nc.vector.tensor_copy(ones_col, ones_col_f32)
    ones_row_f32 = singles.tile([1, P], F32)
    nc.vector.memset(ones_row_f32, 1.0)
    ones_row = singles.tile([1, P], F32R)
    nc.vector.tensor_copy(ones_row, ones_row_f32)

    # c_sq row [1, K] (raw sum of squares of codebook rows)
    csq_sb = singles.tile([1, K], F32R)
    # transposed codebook [d_local, half, k] (raw)
    cbt = cbt_pool.tile([P, DH, K], F32R)

    # ---- preamble: load + transpose codebook, compute c_sq row ----
    with (
        tc.tile_pool(name="pre", bufs=2) as pre,
        tc.tile_pool(name="pre_sq", bufs=1) as pre_sq,
        tc.tile_pool(name="pre_ps", bufs=2, space="PSUM") as pre_ps,
        tc.tile_pool(name="pre_ps_row", bufs=2, space="PSUM") as pre_ps_row,
    ):
        for kt in range(KT):
            cbn = pre.tile([P, D], F32)
            nc.sync.dma_start(out=cbn, in_=cb_in[kt * P : (kt + 1) * P, :])
            for h in range(DH):
                ps_t = pre_ps.tile([P, P], F32)
                nc.tensor.transpose(ps_t, cbn[:, h * P : (h + 1) * P], ident)
                dst = cbt[:, h, kt * P : (kt + 1) * P]
                if (kt + h) % 2 == 0:
                    nc.scalar.copy(dst, ps_t)
                else:
                    nc.vector.tensor_copy(dst, ps_t)

        # squares of cbt, then column-sum via PE with ones weights
        sqc = pre_sq.tile([P, DH, K], F32R)
        nc.scalar.activation(out=sqc[:, 0, :], in_=cbt[:, 0, :], func=AF.Square)
        nc.vector.tensor_mul(sqc[:, 1, :], cbt[:, 1, :], cbt[:, 1, :])
        for kq in range(K // KH):
            ps_c = pre_ps_row.tile([1, KH], F32)
            for h in range(DH):
                nc.tensor.matmul(
                    ps_c,
                    lhsT=ones_col,
                    rhs=sqc[:, h, kq * KH : (kq + 1) * KH],
                    start=(h == 0),
                    stop=(h == DH - 1),
                )
            nc.vector.tensor_copy(csq_sb[:, kq * KH : (kq + 1) * KH], ps_c)

    # ---- main loop over bt tiles ----
    xn_pool = ctx.enter_context(tc.tile_pool(name="xn", bufs=3))
    xt_pool = ctx.enter_context(tc.tile_pool(name="xt", bufs=3))
    dump_pool = ctx.enter_context(tc.tile_pool(name="dump", bufs=2))
    small_pool = ctx.enter_context(tc.tile_pool(name="small", bufs=4))
    out_pool = ctx.enter_context(tc.tile_pool(name="outp", bufs=3))
    ps_t_pool = ctx.enter_context(tc.tile_pool(name="ps_t", bufs=2, space="PSUM"))
    ps_mm_pool = ctx.enter_context(tc.tile_pool(name="ps_mm", bufs=4, space="PSUM"))

    for i in range(NT):
        xn = xn_pool.tile([P, D], F32)
        nc.sync.dma_start(out=xn, in_=x_in[i * P : (i + 1) * P, :])

        # x_sq[bt] = sum_d x^2 via ScalarE square + row-accumulate
        dump = dump_pool.tile([P, D], F32)
        x_sq = small_pool.tile([P, 1], F32)
        nc.scalar.activation(out=dump, in_=xn, func=AF.Square, accum_out=x_sq)
        negp_xsq = small_pool.tile([P, 1], F32)
        nc.vector.tensor_mul(negp_xsq, x_sq, neg_p)

        # transpose x tile, folding in the -2 factor
        xt2 = xt_pool.tile([P, DH, P], F32R)
        for h in range(DH):
            ps_t = ps_t_pool.tile([P, P], F32)
            nc.tensor.transpose(ps_t, xn[:, h * P : (h + 1) * P], ident)
            if h == 0:
                nc.scalar.activation(
                    out=xt2[:, h, :], in_=ps_t, func=AF.Identity, scale=-2.0
                )
            else:
                nc.vector.tensor_scalar_mul(xt2[:, h, :], ps_t, -2.0)

        out_t = out_pool.tile([P, K], F32)
        ps_mms = [
            ps_mm_pool.tile(
                [P, KH], F32, name=f"ps_mm{kq}", tag=f"ps_mm{kq}", bufs=2
            )
            for kq in range(2)
        ]
        for h in range(DH):
            for kq in range(2):
                nc.tensor.matmul(
                    ps_mms[kq],
                    lhsT=xt2[:, h, :],
                    rhs=cbt[:, h, kq * KH : (kq + 1) * KH],
                    start=(h == 0),
                    stop=False,
                )
        for kq in range(2):
            # rank-1: add c_sq to every row
            nc.tensor.matmul(
                ps_mms[kq],
                lhsT=ones_row,
                rhs=csq_sb[:, kq * KH : (kq + 1) * KH],
                start=False,
                stop=True,
            )
        # epilogue: out = neg_p * psum + negp_xsq   (psum = -2 x.c + c_sq)
        nc.scalar.activation(
            out=out_t[:, 0:KH],
            in_=ps_mms[0],
            func=AF.Identity,
            bias=negp_xsq,
            scale=neg_p,
        )
        nc.vector.tensor_scalar(
            out=out_t[:, KH:K],
            in0=ps_mms[1],
            scalar1=neg_p,
            scalar2=negp_xsq,
            op0=OP.mult,
            op1=OP.add,
        )

        nc.sync.dma_start(out=out[i * P : (i + 1) * P, :], in_=out_t)


def build_program():
    nc = bacc.Bacc(
        "TRN2", target_bir_lowering=False, debug=False, num_devices=N_CORES
    )
    x_in = nc.dram_tensor("x", [BT, D], F32, kind="ExternalInput").ap()
    cb_in = nc.dram_tensor("codebook", [K, D], F32, kind="ExternalInput").ap()
    p_in = nc.dram_tensor("precision", [1, 1], F32, kind="ExternalInput").ap()
    out = nc.dram_tensor("out", [BT, K], F32, kind="ExternalOutput").ap()

    with tile.TileContext(nc) as tc:
        with ExitStack() as ctx:
            _build_kernel(ctx, tc, x_in, cb_in, p_in, out)
    nc.compile()
    return nc


_PROGRAM = None


def _get_program():
    global _PROGRAM
    if _PROGRAM is None:
        _PROGRAM = build_program()
    return _PROGRAM


_RESET_DONE = False


def _reset_axon_device():
    """Best-effort terminal-side NRT reset: a previously crashed run can
    leave the NeuronCores in NRT_EXEC_UNIT_UNRECOVERABLE state."""
    global _RESET_DONE
    if _RESET_DONE:
        return
    _RESET_DONE = True
    try:
        import ctypes

        import jax

        jax.devices()  # ensure the PJRT client is initialized
        lib = ctypes.CDLL("/opt/axon/libaxon_pjrt.so")
        lib.axon_reset.restype = ctypes.c_int64
        lib.axon_reset()
    except Exception:
        pass


def kernel(x, codebook, precision, _trace=False):
    x = np.ascontiguousarray(np.asarray(x, dtype=np.float32))
    codebook = np.ascontiguousarray(np.asarray(codebook, dtype=np.float32))
    precision = np.ascontiguousarray(np.asarray(precision, dtype=np.float32))
    assert x.shape == (B, T, D) and codebook.shape == (K, D)

    _reset_axon_device()
    nc = _get_program()
    rows_per_core = B // N_CORES  # 2 batches per core
    in_maps = [
        {
            "x": x[c * rows_per_core : (c + 1) * rows_per_core].reshape(BT, D),
            "codebook": codebook,
            "precision": precision.reshape(1, 1),
        }
        for c in range(N_CORES)
    ]
    res = run_bass_kernel_spmd(
        nc, in_maps, core_ids=list(range(N_CORES)), trace=_trace
    )
    out = np.concatenate(
        [r["out"].reshape(rows_per_core, T, K) for r in res.results], axis=0
    )
    if _trace:
        kernel.last_exec_time_ns = res.exec_time_ns
        kernel.last_results = res
    return out


if __name__ == "__main__":
    xs = np.random.randn(B, T, D).astype(np.float32)
    cb = np.random.randn(K, D).astype(np.float32)
    pr = np.ones((1,), dtype=np.float32)
    o = kernel(xs, cb, pr)
    print(o.shape, o.dtype)



# revision 8
# speedup vs baseline: 1.1592x; 1.1592x over previous
"""Trainium2 Bass kernel: negative squared-distance VQ codebook scores.

score[b,t,k] = -precision * ||x[b,t] - codebook[k]||^2
             = 2p * (x.c) - p * ||x||^2 - p * ||c||^2

Strategy (8 NeuronCores, data-parallel over B):
  - Each core gets 2 batches = 2048 (b,t) rows of x; codebook replicated.
  - GEMM in bf16 (2x fp32r rate, 1024-wide moving operand):
      psum = x.c - 0.5*||c||^2   (rank-1 update with -0.5 row)
    entirely precision-independent; p folds into the epilogue.
  - x transposed on-chip via the DMA crossbar (dma_start_transpose),
    codebook transposed once via the PE.
  - x_sq via fused DVE tensor_tensor_reduce (out=-x^2/2, accum=-xsq/2).
  - Epilogue: out_bf16 = psum * 2p + (-p*xsq)  split ACT/DVE halves.
  - Output stored bf16 (halves HBM traffic), cast to f32 on host.
"""

from contextlib import ExitStack

import numpy as np

import concourse.bass as bass
import concourse.tile as tile
from concourse import bacc, mybir
from concourse.bass_utils import run_bass_kernel_spmd

B, T, D, K = 16, 1024, 256, 1024
N_CORES = 8
BT = B * T // N_CORES  # rows of x per core (2048)
P = 128                # partition tile
NT = BT // P           # bt tiles per core (16)
NPAIR = NT // 2        # paired iterations (8)
KT = K // P            # codebook row tiles (8)
KH = K // 2            # epilogue half (512)

F32 = mybir.dt.float32
BF16 = mybir.dt.bfloat16
AF = mybir.ActivationFunctionType
OP = mybir.AluOpType


def _build_kernel(ctx: ExitStack, tc: tile.TileContext, x_in, cb_in, p_in, out):
    nc = tc.nc

    singles = ctx.enter_context(tc.tile_pool(name="singles", bufs=1))
    cbn_pool = ctx.enter_context(tc.tile_pool(name="cbn", bufs=1))
    cbbf_pool = ctx.enter_context(tc.tile_pool(name="cbbf", bufs=1))
    xn_pool = ctx.enter_context(tc.tile_pool(name="xn", bufs=3))
    xbf_pool = ctx.enter_context(tc.tile_pool(name="xbf", bufs=3))
    xt_pool = ctx.enter_context(tc.tile_pool(name="xt", bufs=3))
    dump_pool = ctx.enter_context(tc.tile_pool(name="dump", bufs=2))
    small_pool = ctx.enter_context(tc.tile_pool(name="small", bufs=8))
    out_pool = ctx.enter_context(tc.tile_pool(name="outp", bufs=2))
    ps_pool = ctx.enter_context(tc.tile_pool(name="ps", bufs=4, space="PSUM"))
    pre_ps_pool = ctx.enter_context(
        tc.tile_pool(name="pre_ps", bufs=2, space="PSUM")
    )
    psc_pool = ctx.enter_context(tc.tile_pool(name="psc", bufs=1, space="PSUM"))

    # ---- tiny setup + first x prefetch, issued before the cb pipeline ----
    p_bc = singles.tile([P, 1], F32)
    nc.sync.dma_start(out=p_bc, in_=p_in.to_broadcast([P, 1]))
    two_p = singles.tile([P, 1], F32)
    nc.scalar.mul(two_p, p_bc, 2.0)  # first ACT op; fires table load early

    # identity (bf16) for PE transposes of the codebook
    ident = singles.tile([P, P], BF16)
    nc.gpsimd.memset(ident, 0.0)
    nc.gpsimd.affine_select(
        out=ident,
        in_=ident,
        compare_op=OP.not_equal,
        fill=1.0,
        base=0,
        pattern=[[-1, P]],
        channel_multiplier=1,
    )
    # rank-1 row of -0.5 (folds the -||c||^2/2 term into the GEMM)
    neghalf = singles.tile([1, P], BF16)
    nc.vector.memset(neghalf, -0.5)

    # first x pair in flight before the codebook loads queue up
    xn_tiles = {}
    xn_tiles[0] = xn_pool.tile([P, 2, D], F32, name="xn0", tag="xn")
    nc.sync.dma_start(
        out=xn_tiles[0],
        in_=x_in[0 : 2 * P, :].rearrange("(j p) d -> p j d", p=P),
    )

    # ---- codebook pipeline ----
    # loads spread across four queues
    cbn = []
    for kt in range(KT):
        t_ = cbn_pool.tile([P, D], F32, name=f"cbn{kt}")
        eng = (nc.sync, nc.sync, nc.scalar, nc.scalar, nc.sync, nc.scalar,
               nc.gpsimd, nc.gpsimd)[kt]
        eng.dma_start(out=t_, in_=cb_in[kt * P : (kt + 1) * P, :])
        cbn.append(t_)

    cbt = singles.tile([P, 2, K], BF16)       # transposed codebook [d, h, k]
    csq_cols = singles.tile([P, KT], F32)     # per-kt ||c||^2 columns
    dump_cb = dump_pool.tile([P, D], BF16, name="dump_cb")

    cbbf = []
    for kt in range(KT):
        # cast f32 -> bf16 (alternate gpsimd / vector)
        cb_bf = cbbf_pool.tile([P, D], BF16, name=f"cbbf{kt}")
        if kt % 2 == 0:
            nc.gpsimd.tensor_copy(cb_bf, cbn[kt])
        else:
            nc.vector.tensor_copy(cb_bf, cbn[kt])
        cbbf.append(cb_bf)
        # raw ||c||^2 per row (f32), alternate ACT square-accum / DVE ttr
        if kt % 2 == 0:
            nc.scalar.activation(
                out=dump_cb, in_=cbn[kt], func=AF.Square,
                accum_out=csq_cols[:, kt : kt + 1],
            )
        else:
            nc.vector.scalar_tensor_tensor(
                out=dump_cb, in0=cbn[kt], scalar=1.0, in1=cbn[kt],
                op0=OP.bypass, op1=OP.mult,
                accum_out=csq_cols[:, kt : kt + 1],
            )
        # PE transpose both d-halves into cbt
        for h in range(2):
            ps_t = pre_ps_pool.tile([P, P], BF16)
            nc.tensor.transpose(ps_t, cb_bf[:, h * P : (h + 1) * P], ident)
            dst = cbt[:, h, kt * P : (kt + 1) * P]
            if (kt + h) % 2 == 0:
                nc.scalar.copy(dst, ps_t)
            else:
                nc.vector.tensor_copy(dst, ps_t)

    # csq row [1, K] bf16: cast cols, PE-transpose [P,KT]->[KT,P], gather row
    csq_cols_bf = singles.tile([P, KT], BF16)
    nc.vector.tensor_copy(csq_cols_bf, csq_cols)
    ps_c = psc_pool.tile([KT, P], BF16)
    nc.tensor.transpose(ps_c, csq_cols_bf, ident)
    s8 = singles.tile([KT, P], BF16)
    nc.scalar.copy(s8, ps_c)
    csqrow = singles.tile([1, K], BF16)
    nc.sync.dma_start(
        out=csqrow.rearrange("a (j q) -> a j q", j=KT), in_=s8
    )

    # ---- main loop over bt tile pairs, software-pipelined emission ----
    xbf_tiles = {}
    xt_tiles = {}
    npxsq = {}

    def emit_pre(i2):
        if i2 not in xn_tiles:
            xn_tiles[i2] = xn_pool.tile([P, 2, D], F32, name=f"xn{i2}", tag="xn")
            nc.sync.dma_start(
                out=xn_tiles[i2],
                in_=x_in[i2 * 2 * P : (i2 + 1) * 2 * P, :].rearrange(
                    "(j p) d -> p j d", p=P
                ),
            )
        xn2 = xn_tiles[i2]
        xbf2 = xbf_pool.tile([P, 2, D], BF16, name=f"xbf{i2}", tag="xb")
        nc.gpsimd.tensor_copy(xbf2, xn2)
        xbf_tiles[i2] = xbf2
        xt2 = xt_pool.tile([P, 4, P], BF16, name=f"xt{i2}", tag="xt")
        nc.sync.dma_start_transpose(out=xt2, in_=xbf2)
        xt_tiles[i2] = xt2
        for j in range(2):
            i = 2 * i2 + j
            dmp = dump_pool.tile([P, D], BF16, name=f"dmp{i}", tag="dmp")
            nh = small_pool.tile([P, 1], F32, name=f"nh{i}", tag="nh")
            nc.vector.scalar_tensor_tensor(
                out=dmp, in0=xn2[:, j, :], scalar=-0.5, in1=xn2[:, j, :],
                op0=OP.mult, op1=OP.mult, accum_out=nh,
            )
            npx = small_pool.tile([P, 1], F32, name=f"npx{i}", tag="npx")
            nc.gpsimd.tensor_scalar_mul(npx, nh, two_p)  # -p * ||x||^2
            npxsq[i] = npx

    def emit_mm_epi(i2):
        xt2 = xt_tiles[i2]
        out2 = out_pool.tile([P, 2, K], BF16, name=f"o{i2}", tag="o")
        for j in range(2):
            i = 2 * i2 + j
            pss = [
                ps_pool.tile([P, KH], F32, name=f"ps{i}_{kq}", tag=f"ps{kq}",
                             bufs=2)
                for kq in range(2)
            ]
            for h in range(2):
                for kq in range(2):
                    nc.tensor.matmul(
                        pss[kq], lhsT=xt2[:, 2 * j + h, :],
                        rhs=cbt[:, h, kq * KH : (kq + 1) * KH],
                        start=(h == 0), stop=False,
                    )
            for kq in range(2):
                nc.tensor.matmul(
                    pss[kq], lhsT=neghalf,
                    rhs=csqrow[:, kq * KH : (kq + 1) * KH],
                    start=False, stop=True,
                )
            # epilogue: out = psum * 2p + (-p * xsq), split ACT / DVE
            nc.scalar.activation(
                out=out2[:, j, 0:KH], in_=pss[0], func=AF.Identity,
                bias=npxsq[i], scale=two_p,
            )
            nc.vector.tensor_scalar(
                out=out2[:, j, KH:K], in0=pss[1],
                scalar1=two_p, scalar2=npxsq[i], op0=OP.mult, op1=OP.add,
            )
        nc.scalar.dma_start(
            out=out[i2 * 2 * P : (i2 + 1) * 2 * P, :].rearrange(
                "(j p) k -> p j k", p=P
            ),
            in_=out2,
        )

    DEPTH = 2
    for i2 in range(min(DEPTH, NPAIR)):
        emit_pre(i2)
    for i2 in range(NPAIR):
        emit_mm_epi(i2)
        if i2 + DEPTH < NPAIR:
            emit_pre(i2 + DEPTH)


def build_program():
    nc = bacc.Bacc(
        "TRN2", target_bir_lowering=False, debug=False, num_devices=N_CORES
    )
    x_in = nc.dram_tensor("x", [BT, D], F32, kind="ExternalInput").ap()
    cb_in = nc.dram_tensor("codebook", [K, D], F32, kind="ExternalInput").ap()
    p_in = nc.dram_tensor("precision", [1, 1], F32, kind="ExternalInput").ap()
    out = nc.dram_tensor("out", [BT, K], BF16, kind="ExternalOutput").ap()

    with tile.TileContext(nc) as tc:
        with ExitStack() as ctx:
            _build_kernel(ctx, tc, x_in, cb_in, p_in, out)
    nc.compile()
    return nc


_PROGRAM = None


def _get_program():
    global _PROGRAM
    if _PROGRAM is None:
        _PROGRAM = build_program()
    return _PROGRAM


_RESET_DONE = False


def _reset_axon_device():
    """Best-effort terminal-side NRT reset: a previously crashed run can
    leave the NeuronCores in NRT_EXEC_UNIT_UNRECOVERABLE state."""
    global _RESET_DONE
    if _RESET_DONE:
        return
    _RESET_DONE = True
    try:
        import ctypes

        import jax

        jax.devices()  # ensure the PJRT client is initialized
        lib = ctypes.CDLL("/opt/axon/libaxon_pjrt.so")
        lib.axon_reset.restype = ctypes.c_int64
        lib.axon_reset()
    except Exception:
        pass


def kernel(x, codebook, precision, _trace=False):
    x = np.ascontiguousarray(np.asarray(x, dtype=np.float32))
    codebook = np.ascontiguousarray(np.asarray(codebook, dtype=np.float32))
    precision = np.ascontiguousarray(np.asarray(precision, dtype=np.float32))
    assert x.shape == (B, T, D) and codebook.shape == (K, D)

    _reset_axon_device()
    nc = _get_program()
    rows_per_core = B // N_CORES  # 2 batches per core
    in_maps = [
        {
            "x": x[c * rows_per_core : (c + 1) * rows_per_core].reshape(BT, D),
            "codebook": codebook,
            "precision": precision.reshape(1, 1),
        }
        for c in range(N_CORES)
    ]
    res = run_bass_kernel_spmd(
        nc, in_maps, core_ids=list(range(N_CORES)), trace=_trace
    )
    out = np.concatenate(
        [
            np.asarray(r["out"]).astype(np.float32).reshape(rows_per_core, T, K)
            for r in res.results
        ],
        axis=0,
    )
    if _trace:
        kernel.last_exec_time_ns = res.exec_time_ns
        kernel.last_results = res
    return out


if __name__ == "__main__":
    xs = np.random.randn(B, T, D).astype(np.float32)
    cb = np.random.randn(K, D).astype(np.float32)
    pr = np.ones((1,), dtype=np.float32)
    o = kernel(xs, cb, pr)
    print(o.shape, o.dtype)


# revision 10
# speedup vs baseline: 1.1655x; 1.0054x over previous
"""Trainium2 Bass kernel: negative squared-distance VQ codebook scores.

score[b,t,k] = -precision * ||x[b,t] - codebook[k]||^2
             = 2p * (x.c) - p * ||x||^2 - p * ||c||^2

Strategy (8 NeuronCores, data-parallel over B):
  - Each core gets 2 batches = 2048 (b,t) rows of x; codebook replicated.
  - GEMM in bf16: psum = x.c - 0.5*||c||^2 (rank-1 update, p-independent).
  - x and codebook transposed via the DMA crossbar (dma_start_transpose);
    codebook cast writes an h-major layout so one big transpose suffices.
  - ||c||^2 from squares of the transposed codebook + ones-column matmuls.
  - Epilogue: out_bf16 = psum * 2p + (-p*||x||^2), ACT/DVE halves.
  - Output stored bf16 (halves HBM traffic), cast to f32 on host.
"""

from contextlib import ExitStack

import numpy as np

import concourse.bass as bass
import concourse.tile as tile
from concourse import bacc, mybir
from concourse.bass_utils import run_bass_kernel_spmd

B, T, D, K = 16, 1024, 256, 1024
N_CORES = 8
BT = B * T // N_CORES  # rows of x per core (2048)
P = 128                # partition tile
NT = BT // P           # bt tiles per core (16)
NPAIR = NT // 2        # paired iterations (8)
KT = K // P            # codebook row tiles (8)
KH = K // 2            # epilogue half (512)

F32 = mybir.dt.float32
BF16 = mybir.dt.bfloat16
AF = mybir.ActivationFunctionType
OP = mybir.AluOpType


def _build_kernel(ctx: ExitStack, tc: tile.TileContext, x_in, cb_in, p_in, out):
    nc = tc.nc

    singles = ctx.enter_context(tc.tile_pool(name="singles", bufs=1))
    xn_pool = ctx.enter_context(tc.tile_pool(name="xn", bufs=3))
    xbf_pool = ctx.enter_context(tc.tile_pool(name="xbf", bufs=3))
    xt_pool = ctx.enter_context(tc.tile_pool(name="xt", bufs=3))
    dump_pool = ctx.enter_context(tc.tile_pool(name="dump", bufs=2))
    small_pool = ctx.enter_context(tc.tile_pool(name="small", bufs=8))
    out_pool = ctx.enter_context(tc.tile_pool(name="outp", bufs=2))
    ps_pool = ctx.enter_context(tc.tile_pool(name="ps", bufs=4, space="PSUM"))
    psc_pool = ctx.enter_context(tc.tile_pool(name="psc", bufs=2, space="PSUM"))

    # ---- tiny setup + first x prefetch before the cb pipeline queues up ----
    p_bc = singles.tile([P, 1], F32)
    nc.sync.dma_start(out=p_bc, in_=p_in.to_broadcast([P, 1]))
    two_p = singles.tile([P, 1], F32)
    nc.scalar.mul(two_p, p_bc, 2.0)  # first ACT op; fires table load early

    neghalf = singles.tile([1, P], BF16)   # rank-1 row: -0.5 * ||c||^2
    nc.vector.memset(neghalf, -0.5)
    ones_col = singles.tile([P, 1], BF16)  # column-sum weights for ||c||^2
    nc.vector.memset(ones_col, 1.0)

    xn_tiles = {}

    def load_xn(i2):
        t = xn_pool.tile([P, 2, D], F32, name=f"xn{i2}", tag="xn")
        nc.sync.dma_start(
            out=t,
            in_=x_in[i2 * 2 * P : (i2 + 1) * 2 * P, :].rearrange(
                "(j p) d -> p j d", p=P
            ),
        )
        xn_tiles[i2] = t

    load_xn(0)

    # ---- codebook pipeline ----
    # two paired loads each on sync and scalar queues
    cbn = singles.tile([P, 4, 2, D], F32)  # [p, pair, j, d]; row = pr*256+j*128+p
    for pr in range(4):
        eng = (nc.sync, nc.scalar, nc.sync, nc.scalar)[pr]
        eng.dma_start(
            out=cbn[:, pr, :, :],
            in_=cb_in[pr * 2 * P : (pr + 1) * 2 * P, :].rearrange(
                "(j p) d -> p j d", p=P
            ),
        )

    # cast to bf16 in h-major layout: cbbf[q, h, kt, pd] = cb[kt*128+q, h*128+pd]
    cbbf = singles.tile([P, 2, KT, P], BF16)
    # first x cast must beat the cb casts on DVE (xt2(0) depends on it)
    xbf0 = xbf_pool.tile([P, 2, D], BF16, name="xbf0", tag="xb")
    nc.vector.tensor_copy(xbf0, xn_tiles[0])
    xbf_tiles = {0: xbf0}
    for kt in range(KT):
        src = cbn[:, kt // 2, kt % 2, :].rearrange("q (h pd) -> q h pd", h=2)
        dst = cbbf[:, :, kt, :]
        if kt % 2 == 0:
            nc.scalar.copy(dst, src)
        else:
            nc.vector.tensor_copy(dst, src)

    # one crossbar transpose: cbt[pd_, jj, q] = cbbf_flat[q, jj*128 + pd_]
    # with jj = h*8 + kt, so cbt[:, h*8+kt, :] holds d-half h of k-tile kt.
    cbt = singles.tile([P, 2 * KT, P], BF16)
    nc.sync.dma_start_transpose(out=cbt, in_=cbbf.rearrange("q h kt pd -> q (h kt pd)"))

    def cbt_h(h, kq):  # [128, 512] moving operand: d-half h, k columns kq*512..
        return cbt[:, h * KT + kq * 4 : h * KT + (kq + 1) * 4, :]

    sqc = singles.tile([P, 2 * KT, P], BF16)
    csqrow = singles.tile([1, K], BF16)

    # ---- main loop over bt tile pairs, software-pipelined emission ----
    xt_tiles = {}
    npxsq = {}

    def emit_pre(i2):
        if i2 not in xn_tiles:
            load_xn(i2)
        xn2 = xn_tiles[i2]
        if i2 in xbf_tiles:
            xbf2 = xbf_tiles[i2]
        else:
            xbf2 = xbf_pool.tile([P, 2, D], BF16, name=f"xbf{i2}", tag="xb")
            nc.vector.tensor_copy(xbf2, xn2)
            xbf_tiles[i2] = xbf2
        xt2 = xt_pool.tile([P, 4, P], BF16, name=f"xt{i2}", tag="xt")
        nc.sync.dma_start_transpose(out=xt2, in_=xbf2)
        xt_tiles[i2] = xt2
        for j in range(2):
            i = 2 * i2 + j
            dmp = dump_pool.tile([P, D], BF16, name=f"dmp{i}", tag="dmp")
            xsq = small_pool.tile([P, 1], F32, name=f"xsq{i}", tag="xsq")
            nc.scalar.activation(
                out=dmp, in_=xn2[:, j, :], func=AF.Square, accum_out=xsq
            )
            npx = small_pool.tile([P, 1], F32, name=f"npx{i}", tag="npx")
            # -p * ||x||^2 = (xsq * 2p) * -0.5
            nc.gpsimd.tensor_scalar(
                out=npx, in0=xsq, scalar1=two_p, scalar2=-0.5,
                op0=OP.mult, op1=OP.mult,
            )
            npxsq[i] = npx

    def emit_mm_epi(i2):
        xt2 = xt_tiles[i2]
        out2 = out_pool.tile([P, 2, K], BF16, name=f"o{i2}", tag="o")
        for j in range(2):
            i = 2 * i2 + j
            pss = [
                ps_pool.tile([P, KH], F32, name=f"ps{i}_{kq}", tag=f"ps{kq}",
                             bufs=2)
                for kq in range(2)
            ]
            for h in range(2):
                for kq in range(2):
                    nc.tensor.matmul(
                        pss[kq], lhsT=xt2[:, 2 * j + h, :], rhs=cbt_h(h, kq),
                        start=(h == 0), stop=False,
                    )
            for kq in range(2):
                nc.tensor.matmul(
                    pss[kq], lhsT=neghalf,
                    rhs=csqrow[:, kq * KH : (kq + 1) * KH],
                    start=False, stop=True,
                )
            # epilogue: out = psum * 2p + (-p * xsq), split ACT / DVE
            nc.scalar.activation(
                out=out2[:, j, 0:KH], in_=pss[0], func=AF.Identity,
                bias=npxsq[i], scale=two_p,
            )
            nc.vector.tensor_scalar(
                out=out2[:, j, KH:K], in0=pss[1],
                scalar1=two_p, scalar2=npxsq[i], op0=OP.mult, op1=OP.add,
            )
        nc.gpsimd.dma_start(
            out=out[i2 * 2 * P : (i2 + 1) * 2 * P, :].rearrange(
                "(j p) k -> p j k", p=P
            ),
            in_=out2,
        )

    def emit_csq():
        nc.scalar.activation(
            out=sqc[:, 0:KT, :], in_=cbt[:, 0:KT, :], func=AF.Square
        )
        nc.vector.tensor_mul(sqc[:, KT:, :], cbt[:, KT:, :], cbt[:, KT:, :])
        for kq in range(2):
            ps_c = psc_pool.tile([1, KH], F32)
            for h in range(2):
                nc.tensor.matmul(
                    ps_c,
                    lhsT=ones_col,
                    rhs=sqc[:, h * KT + kq * 4 : h * KT + (kq + 1) * 4, :],
                    start=(h == 0),
                    stop=(h == 1),
                )
            if kq == 0:
                nc.scalar.copy(csqrow[:, 0:KH], ps_c)
            else:
                nc.vector.tensor_copy(csqrow[:, KH:K], ps_c)

    DEPTH = 3
    for i2 in range(min(DEPTH, NPAIR)):
        emit_pre(i2)
    emit_csq()
    for i2 in range(NPAIR):
        emit_mm_epi(i2)
        if i2 + DEPTH < NPAIR:
            emit_pre(i2 + DEPTH)


def build_program():
    nc = bacc.Bacc(
        "TRN2", target_bir_lowering=False, debug=False, num_devices=N_CORES
    )
    x_in = nc.dram_tensor("x", [BT, D], F32, kind="ExternalInput").ap()
    cb_in = nc.dram_tensor("codebook", [K, D], F32, kind="ExternalInput").ap()
    p_in = nc.dram_tensor("precision", [1, 1], F32, kind="ExternalInput").ap()
    out = nc.dram_tensor("out", [BT, K], BF16, kind="ExternalOutput").ap()

    with tile.TileContext(nc) as tc:
        with ExitStack() as ctx:
            _build_kernel(ctx, tc, x_in, cb_in, p_in, out)
    nc.compile()
    return nc


_PROGRAM = None


def _get_program():
    global _PROGRAM
    if _PROGRAM is None:
        _PROGRAM = build_program()
    return _PROGRAM


_RESET_DONE = False


def _reset_axon_device():
    """Best-effort terminal-side NRT reset: a previously crashed run can
    leave the NeuronCores in NRT_EXEC_UNIT_UNRECOVERABLE state."""
    global _RESET_DONE
    if _RESET_DONE:
        return
    _RESET_DONE = True
    try:
        import ctypes

        import jax

        jax.devices()  # ensure the PJRT client is initialized
        lib = ctypes.CDLL("/opt/axon/libaxon_pjrt.so")
        lib.axon_reset.restype = ctypes.c_int64
        lib.axon_reset()
    except Exception:
        pass


def kernel(x, codebook, precision, _trace=False):
    x = np.ascontiguousarray(np.asarray(x, dtype=np.float32))
    codebook = np.ascontiguousarray(np.asarray(codebook, dtype=np.float32))
    precision = np.ascontiguousarray(np.asarray(precision, dtype=np.float32))
    assert x.shape == (B, T, D) and codebook.shape == (K, D)

    _reset_axon_device()
    nc = _get_program()
    rows_per_core = B // N_CORES  # 2 batches per core
    in_maps = [
        {
            "x": x[c * rows_per_core : (c + 1) * rows_per_core].reshape(BT, D),
            "codebook": codebook,
            "precision": precision.reshape(1, 1),
        }
        for c in range(N_CORES)
    ]
    res = run_bass_kernel_spmd(
        nc, in_maps, core_ids=list(range(N_CORES)), trace=_trace
    )
    out = np.concatenate(
        [
            np.asarray(r["out"]).astype(np.float32).reshape(rows_per_core, T, K)
            for r in res.results
        ],
        axis=0,
    )
    if _trace:
        kernel.last_exec_time_ns = res.exec_time_ns
        kernel.last_results = res
    return out


if __name__ == "__main__":
    xs = np.random.randn(B, T, D).astype(np.float32)
    cb = np.random.randn(K, D).astype(np.float32)
    pr = np.ones((1,), dtype=np.float32)
    o = kernel(xs, cb, pr)
    print(o.shape, o.dtype)


# revision 11
# speedup vs baseline: 1.1823x; 1.0144x over previous
"""Trainium2 Bass kernel: negative squared-distance VQ codebook scores.

score[b,t,k] = -precision * ||x[b,t] - codebook[k]||^2
             = 2p * (x.c) - p * ||x||^2 - p * ||c||^2

Strategy (8 NeuronCores, data-parallel over B):
  - Each core gets 2 batches = 2048 (b,t) rows of x; codebook replicated.
  - GEMM in bf16: psum = x.c - 0.5*||c||^2 (rank-1 update, p-independent).
  - x processed in groups of 4 row-tiles: one load, one cast, one crossbar
    transpose per group (dma_start_transpose has ~1.3us fixed cost).
  - Codebook cast writes an h-major layout so one crossbar transpose
    suffices; ||c||^2 from squares of the transposed codebook + ones
    matmuls.
  - Epilogue: out_bf16 = psum * 2p + (-p*||x||^2), ACT/DVE halves.
  - Output stored bf16 (halves HBM traffic), cast to f32 on host.
"""

from contextlib import ExitStack

import numpy as np

import concourse.bass as bass
import concourse.tile as tile
from concourse import bacc, mybir
from concourse.bass_utils import run_bass_kernel_spmd

B, T, D, K = 16, 1024, 256, 1024
N_CORES = 8
BT = B * T // N_CORES  # rows of x per core (2048)
P = 128                # partition tile
NT = BT // P           # bt tiles per core (16)
GT = 4                 # tiles per x group
NG = NT // GT          # x groups (4)
KT = K // P            # codebook column tiles (8)
KH = K // 2            # epilogue half (512)

F32 = mybir.dt.float32
BF16 = mybir.dt.bfloat16
AF = mybir.ActivationFunctionType
OP = mybir.AluOpType


def _build_kernel(ctx: ExitStack, tc: tile.TileContext, x_in, cb_in, p_in, out):
    nc = tc.nc

    singles = ctx.enter_context(tc.tile_pool(name="singles", bufs=1))
    xn_pool = ctx.enter_context(tc.tile_pool(name="xn", bufs=3))
    xbf_pool = ctx.enter_context(tc.tile_pool(name="xbf", bufs=3))
    xt_pool = ctx.enter_context(tc.tile_pool(name="xt", bufs=3))
    dump_pool = ctx.enter_context(tc.tile_pool(name="dump", bufs=2))
    small_pool = ctx.enter_context(tc.tile_pool(name="small", bufs=8))
    out_pool = ctx.enter_context(tc.tile_pool(name="outp", bufs=2))
    ps_pool = ctx.enter_context(tc.tile_pool(name="ps", bufs=4, space="PSUM"))
    psc_pool = ctx.enter_context(tc.tile_pool(name="psc", bufs=2, space="PSUM"))

    # p broadcast on the gpsimd queue (keeps sync/scalar free for the
    # time-critical loads)
    p_bc = singles.tile([P, 1], F32)
    nc.gpsimd.dma_start(out=p_bc, in_=p_in.to_broadcast([P, 1]))

    # ---- x group loads on sync; codebook loads on scalar ----
    xn_tiles = {}

    def load_xg(g):
        t = xn_pool.tile([P, GT, D], F32, name=f"xn{g}", tag="xn")
        nc.sync.dma_start(
            out=t,
            in_=x_in[g * GT * P : (g + 1) * GT * P, :].rearrange(
                "(j p) d -> p j d", p=P
            ),
        )
        xn_tiles[g] = t

    load_xg(0)
    load_xg(1)

    cbn = singles.tile([P, 2, 4, D], F32)  # [p, half, j, d]; k-tile = 4*half+j
    for hl in range(2):
        nc.scalar.dma_start(
            out=cbn[:, hl, :, :],
            in_=cb_in[hl * 4 * P : (hl + 1) * 4 * P, :].rearrange(
                "(j p) d -> p j d", p=P
            ),
        )

    # ---- small constants ----
    two_p = singles.tile([P, 1], F32)
    nc.scalar.mul(two_p, p_bc, 2.0)  # first ACT op; fires table load
    neghalf = singles.tile([1, P], BF16)   # rank-1 row scale: -0.5
    nc.vector.memset(neghalf, -0.5)
    ones_col = singles.tile([P, 1], BF16)  # column-sum weights for ||c||^2
    nc.vector.memset(ones_col, 1.0)

    # ---- per-group x pipeline pieces ----
    xbf_tiles, xt_tiles, npxsq = {}, {}, {}

    def emit_cast(g):
        xbf2 = xbf_pool.tile([P, GT, D], BF16, name=f"xbf{g}", tag="xb")
        nc.vector.tensor_copy(xbf2, xn_tiles[g])
        xbf_tiles[g] = xbf2

    def emit_trans(g):
        xt2 = xt_pool.tile([P, 2 * GT, P], BF16, name=f"xt{g}", tag="xt")
        nc.sync.dma_start_transpose(out=xt2, in_=xbf_tiles[g])
        xt_tiles[g] = xt2

    def emit_xsq(i):
        g, t_ = i // GT, i % GT
        dmp = dump_pool.tile([P, D], BF16, name=f"dmp{i}", tag="dmp")
        xsq = small_pool.tile([P, 1], F32, name=f"xsq{i}", tag="xsq")
        nc.scalar.activation(
            out=dmp, in_=xn_tiles[g][:, t_, :], func=AF.Square, accum_out=xsq
        )
        npx = small_pool.tile([P, 1], F32, name=f"npx{i}", tag="npx")
        nc.gpsimd.tensor_scalar(
            out=npx, in0=xsq, scalar1=two_p, scalar2=-0.5,
            op0=OP.mult, op1=OP.mult,
        )
        npxsq[i] = npx

    # first x group: cast early (ahead of cb casts on DVE), transpose, xsq
    emit_cast(0)
    emit_trans(0)

    # ---- codebook cast (h-major) + one crossbar transpose ----
    # cbbf[q, h, kt, pd] = cb[kt*128+q, h*128+pd]
    cbbf = singles.tile([P, 2, KT, P], BF16)
    for kt in range(KT):
        src = cbn[:, kt // 4, kt % 4, :].rearrange("q (h pd) -> q h pd", h=2)
        dst = cbbf[:, :, kt, :]
        if kt % 2 == 0:
            nc.scalar.copy(dst, src)
        else:
            nc.vector.tensor_copy(dst, src)
    # cbt[pd, h*8+kt, q] = cb[kt*128+q, h*128+pd]
    cbt = singles.tile([P, 2 * KT, P], BF16)
    nc.sync.dma_start_transpose(
        out=cbt, in_=cbbf.rearrange("q h kt pd -> q (h kt pd)")
    )

    def cbt_h(h, kq):  # [128, 512] moving operand: d-half h, k cols kq*512..
        return cbt[:, h * KT + kq * 4 : h * KT + (kq + 1) * 4, :]

    for i in range(GT):
        emit_xsq(i)
    emit_cast(1)
    emit_trans(1)

    # ---- ||c||^2 row from squares of cbt ----
    sqc = singles.tile([P, 2 * KT, P], BF16)
    csqrow = singles.tile([1, K], BF16)
    nc.scalar.activation(out=sqc[:, 0:KT, :], in_=cbt[:, 0:KT, :],
                         func=AF.Square)
    nc.vector.tensor_mul(sqc[:, KT:, :], cbt[:, KT:, :], cbt[:, KT:, :])
    for kq in range(2):
        ps_c = psc_pool.tile([1, KH], F32)
        for h in range(2):
            nc.tensor.matmul(
                ps_c, lhsT=ones_col,
                rhs=sqc[:, h * KT + kq * 4 : h * KT + (kq + 1) * 4, :],
                start=(h == 0), stop=(h == 1),
            )
        if kq == 0:
            nc.scalar.copy(csqrow[:, 0:KH], ps_c)
        else:
            nc.vector.tensor_copy(csqrow[:, KH:K], ps_c)

    # ---- main loop: per tile, with per-tile interleaved prefetch ----
    out_tiles = {}

    def emit_mm_epi(i):
        g, t_ = i // GT, i % GT
        xt2 = xt_tiles[g]
        if t_ == 0:
            out_tiles[g] = out_pool.tile([P, GT, K], BF16, name=f"o{g}",
                                         tag="o")
        out2 = out_tiles[g]
        pss = [
            ps_pool.tile([P, KH], F32, name=f"ps{i}_{kq}", tag=f"ps{kq}",
                         bufs=2)
            for kq in range(2)
        ]
        for h in range(2):
            for kq in range(2):
                nc.tensor.matmul(
                    pss[kq], lhsT=xt2[:, 2 * t_ + h, :], rhs=cbt_h(h, kq),
                    start=(h == 0), stop=False,
                )
        for kq in range(2):
            nc.tensor.matmul(
                pss[kq], lhsT=neghalf,
                rhs=csqrow[:, kq * KH : (kq + 1) * KH],
                start=False, stop=True,
            )
        nc.scalar.activation(
            out=out2[:, t_, 0:KH], in_=pss[0], func=AF.Identity,
            bias=npxsq[i], scale=two_p,
        )
        nc.vector.tensor_scalar(
            out=out2[:, t_, KH:K], in0=pss[1],
            scalar1=two_p, scalar2=npxsq[i], op0=OP.mult, op1=OP.add,
        )
        if t_ == GT - 1:
            nc.sync.dma_start(
                out=out[g * GT * P : (g + 1) * GT * P, :].rearrange(
                    "(j p) k -> p j k", p=P
                ),
                in_=out2,
            )

    for i in range(NT):
        # prefetch: keep 1.5-2 groups in flight
        pf = i + 2 * GT  # tile index 8 ahead
        if pf < NT:
            g = pf // GT
            if pf % GT == 0:
                load_xg(g)
            elif pf % GT == 1:
                emit_cast(g)
            elif pf % GT == 2:
                emit_trans(g)
        if i + GT < NT:
            emit_xsq(i + GT)
        emit_mm_epi(i)


def build_program():
    nc = bacc.Bacc(
        "TRN2", target_bir_lowering=False, debug=False, num_devices=N_CORES
    )
    x_in = nc.dram_tensor("x", [BT, D], F32, kind="ExternalInput").ap()
    cb_in = nc.dram_tensor("codebook", [K, D], F32, kind="ExternalInput").ap()
    p_in = nc.dram_tensor("precision", [1, 1], F32, kind="ExternalInput").ap()
    out = nc.dram_tensor("out", [BT, K], BF16, kind="ExternalOutput").ap()

    with tile.TileContext(nc) as tc:
        with ExitStack() as ctx:
            _build_kernel(ctx, tc, x_in, cb_in, p_in, out)
    nc.compile()
    return nc


_PROGRAM = None


def _get_program():
    global _PROGRAM
    if _PROGRAM is None:
        _PROGRAM = build_program()
    return _PROGRAM


_RESET_DONE = False


def _reset_axon_device():
    """Best-effort terminal-side NRT reset: a previously crashed run can
    leave the NeuronCores in NRT_EXEC_UNIT_UNRECOVERABLE state."""
    global _RESET_DONE
    if _RESET_DONE:
        return
    _RESET_DONE = True
    try:
        import ctypes

        import jax

        jax.devices()  # ensure the PJRT client is initialized
        lib = ctypes.CDLL("/opt/axon/libaxon_pjrt.so")
        lib.axon_reset.restype = ctypes.c_int64
        lib.axon_reset()
    except Exception:
        pass


def kernel(x, codebook, precision, _trace=False):
    x = np.ascontiguousarray(np.asarray(x, dtype=np.float32))
    codebook = np.ascontiguousarray(np.asarray(codebook, dtype=np.float32))
    precision = np.ascontiguousarray(np.asarray(precision, dtype=np.float32))
    assert x.shape == (B, T, D) and codebook.shape == (K, D)

    _reset_axon_device()
    nc = _get_program()
    rows_per_core = B // N_CORES  # 2 batches per core
    in_maps = [
        {
            "x": x[c * rows_per_core : (c + 1) * rows_per_core].reshape(BT, D),
            "codebook": codebook,
            "precision": precision.reshape(1, 1),
        }
        for c in range(N_CORES)
    ]
    res = run_bass_kernel_spmd(
        nc, in_maps, core_ids=list(range(N_CORES)), trace=_trace
    )
    out = np.concatenate(
        [
            np.asarray(r["out"]).astype(np.float32).reshape(rows_per_core, T, K)
            for r in res.results
        ],
        axis=0,
    )
    if _trace:
        kernel.last_exec_time_ns = res.exec_time_ns
        kernel.last_results = res
    return out


if __name__ == "__main__":
    xs = np.random.randn(B, T, D).astype(np.float32)
    cb = np.random.randn(K, D).astype(np.float32)
    pr = np.ones((1,), dtype=np.float32)
    o = kernel(xs, cb, pr)
    print(o.shape, o.dtype)
